# revision 14
# baseline (speedup 1.0000x reference)
"""Additive attention kernel for Trainium2 (8 NeuronCores, data-parallel over batch).

Math: out, attn = softmax_masked(einsum('bqkh,h', tanh(qWq[:,None]+kWk[None,:]), wv)) @ v

The O(B*NQ*NKV*H) tanh is evaluated via a separable sine expansion:
    tanh(x) ~= sum_m beta_m sin(w_m x),  w_m = base * 2^level  (octave ladder)
    sin(w(a+b)) = sin(wa)cos(wb) + cos(wa)sin(wb)
so scores become a sum of 2R rank-H matmuls on the tensor engine; the
per-element transcendentals run only on the small projected tensors.
Base sin/cos come from the ScalarE LUT (args stay within [-pi, pi]);
higher octaves use s2=s*c, c2=1-2s^2 (double angle) with power-of-two
scale factors folded into the per-term coefficient columns. Harmonics are
kept in bf16 (fast DVE modes + PE fast weight load); the -1e6 mask addend
rides in as one extra K=1 matmul term per batch.
"""

import sys

for _p in ("/opt/trn_rl_repo",):
    if _p not in sys.path:
        sys.path.insert(0, _p)

import numpy as np
import ml_dtypes

import concourse.bass as bass
import concourse.bacc as bacc
import concourse.tile as tile
from concourse import mybir
from concourse.bass_utils import run_bass_kernel_spmd

F32 = mybir.dt.float32
F32R = mybir.dt.float32r
BF16 = mybir.dt.bfloat16
AF = mybir.ActivationFunctionType
ALU = mybir.AluOpType

B, NQ, NKV = 16, 128, 512
QS, KS, VS, H = 256, 256, 256, 128
NCORES = 8
BPC = B // NCORES  # batches per core

# sine-ladder fit of tanh (Gaussian-weighted lstsq, see module docstring)
BASES = [0.2110214, 0.29999999]
LEVELS = [5, 5]
BETA = [
    0.89393810279, 0.21792006157, 0.20982351355, 0.08374484344, 0.011219404929,
    0.19989137427, 0.090749504641, 0.12138348077, 0.038702849552, 0.0018794616986,
]
R = sum(LEVELS)
HALF_PI = 1.5707963267948966
MASK_NEG = -60000.0  # large negative addend; exp(score + MASK_NEG) == 0 in fp32


def _sigma_list():
    """Per-frequency power-of-two factor: true s = sigma * stored s~."""
    sig = []
    for L in LEVELS:
        s = 1.0
        for _ in range(L):
            sig.append(s)
            s *= 2.0
    return sig


def build_nc():
    nc = bacc.Bacc(
        "TRN2", target_bir_lowering=False, debug=False, num_devices=NCORES
    )

    qT = nc.declare_dram_parameter("qT", [BPC, QS, NQ], F32R, False)
    kT = nc.declare_dram_parameter("kT", [BPC, KS, NKV], F32R, False)
    vals = nc.declare_dram_parameter("vals", [BPC, NKV, VS], F32R, False)
    Wq = nc.declare_dram_parameter("Wq", [QS, H], F32R, False)
    Wk = nc.declare_dram_parameter("Wk", [KS, H], F32R, False)
    cols = nc.declare_dram_parameter("cols", [H, R + 1], F32, False)
    mrow = nc.declare_dram_parameter("mrow", [1, BPC, NKV], BF16, False)
    mneg = nc.declare_dram_parameter("mneg", [1, NQ], BF16, False)
    ident = nc.declare_dram_parameter("ident", [128, 128], F32, False)
    out = nc.declare_dram_parameter("out", [BPC, NQ, VS], F32, isOutput=True)
    attn = nc.declare_dram_parameter("attn", [BPC, NQ, NKV], F32, isOutput=True)

    FDQ = BPC * NQ      # 256: fused q free dim [b, q]
    FDK = BPC * NKV     # 1024: fused k free dim [b, k]

    with tile.TileContext(nc) as tc:
        with (
            tc.tile_pool(name="const", bufs=1) as cp,
            tc.tile_pool(name="io", bufs=1) as iop,
            tc.tile_pool(name="harm", bufs=1) as hp,
            tc.tile_pool(name="tmp", bufs=3) as tp,
            tc.tile_pool(name="soft", bufs=2) as sp,
            tc.tile_pool(name="ps_proj", bufs=1, space="PSUM") as ps_proj,
            tc.tile_pool(name="ps_sc", bufs=1, space="PSUM") as ps_sc,
            tc.tile_pool(name="ps_t", bufs=1, space="PSUM") as ps_t,
        ):
            # ---------------- constants & inputs ----------------
            wq_sb = cp.tile([128, 2, H], F32R)
            wk_sb = cp.tile([128, 2, H], F32R)
            nc.sync.dma_start(wq_sb[:], Wq.rearrange("(c p) h -> p c h", p=128))
            nc.sync.dma_start(wk_sb[:], Wk.rearrange("(c p) h -> p c h", p=128))
            cols_sb = cp.tile([128, R + 1], F32)
            nc.scalar.dma_start(cols_sb[:], cols[:, :])
            ident_sb = cp.tile([128, 128], F32)
            nc.gpsimd.dma_start(ident_sb[:], ident[:, :])
            mrow_sb = cp.tile([1, BPC, NKV], BF16)
            nc.scalar.dma_start(mrow_sb[:], mrow[:, :, :])
            mneg_sb = cp.tile([1, NQ], BF16)
            nc.scalar.dma_start(mneg_sb[:], mneg[:, :])

            q_in = iop.tile([128, 2, BPC, NQ], F32R)   # [d-chunk][batch][q]
            k_in = iop.tile([128, 2, BPC, NKV], F32R)  # [d-chunk][batch][k]
            for b in range(BPC):
                nc.sync.dma_start(
                    q_in[:, :, b, :], qT[b].rearrange("(c p) q -> p c q", p=128))
                nc.sync.dma_start(
                    k_in[:, :, b, :], kT[b].rearrange("(c p) k -> p c k", p=128))
            v_sb = iop.tile([128, BPC, 4, VS], F32R)   # [batch][k-chunk][v]
            for b in range(BPC):
                nc.gpsimd.dma_start(
                    v_sb[:, b, :, :], vals[b].rearrange("(c p) v -> p c v", p=128))

            # ---------------- projections: theta = x @ W ----------------
            thq_ps = ps_proj.tile([128, FDQ], F32)
            for c in range(2):
                nc.tensor.matmul(
                    thq_ps[:], wq_sb[:, c, :], q_in[:, c],
                    start=(c == 0), stop=(c == 1),
                )
            thq = iop.tile([128, FDQ], F32)
            nc.vector.tensor_copy(thq[:], thq_ps[:])

            thk_ps = ps_proj.tile([128, FDK], F32)
            for b in range(BPC):
                for c in range(2):
                    nc.tensor.matmul(
                        thk_ps[:, b * NKV:(b + 1) * NKV],
                        wk_sb[:, c, :], k_in[:, c, b],
                        start=(c == 0), stop=(c == 1),
                    )
            thk = iop.tile([128, FDK], F32)
            nc.vector.tensor_copy(thk[:], thk_ps[:])

            # ---------------- sine octave ladders (bf16 harmonics) --------
            def ladder(theta, FD, tag):
                tiles = []
                for bi, (nu, L) in enumerate(zip(BASES, LEVELS)):
                    s = hp.tile([128, FD], BF16, tag=f"{tag}s{bi}_0")
                    c = hp.tile([128, FD], BF16, tag=f"{tag}c{bi}_0")
                    nc.scalar.activation(s[:], theta[:], AF.Sin, scale=float(nu))
                    nc.scalar.activation(c[:], theta[:], AF.Sin,
                                         bias=cols_sb[:, R:R + 1],
                                         scale=float(nu))
                    tiles.append((s, c))
                    sigma = 1.0
                    for j in range(1, L):
                        spv, cpv = tiles[-1]
                        sq = tp.tile([128, FD], BF16, tag=f"{tag}sq")
                        nc.scalar.activation(sq[:], spv[:], AF.Square)
                        s2 = hp.tile([128, FD], BF16, tag=f"{tag}s{bi}_{j}")
                        nc.vector.tensor_mul(s2[:], spv[:], cpv[:])
                        c2 = hp.tile([128, FD], BF16, tag=f"{tag}c{bi}_{j}")
                        nc.vector.tensor_scalar(
                            c2[:], sq[:], -2.0 * sigma * sigma, 1.0,
                            ALU.mult, ALU.add,
                        )
                        sigma *= 2.0
                        tiles.append((s2, c2))
                return tiles

            # q-side first so the A-scalings (lhsT producers) run early on DVE
            qh = ladder(thq, FDQ, "q")

            # HAM warmup: junk matmuls on the first q-harmonic keep PE busy
            # from the ladder phase into the scores phase, so the scores
            # matmuls run at 2.4 GHz instead of the cold 1.2 GHz.
            warm_ps = ps_t.tile([128, NKV], F32, tag="warm")
            for w in range(20):
                nc.tensor.matmul(
                    warm_ps[:, 0:FDQ], qh[0][0][:, 0:128], qh[0][0][:],
                    start=True, stop=True, skip_group_check=True,
                )

            a1s, a2s = [], []
            for i in range(R):
                sqt, cqt = qh[i]
                a1 = hp.tile([128, FDQ], BF16, tag=f"a1_{i}")
                a2 = hp.tile([128, FDQ], BF16, tag=f"a2_{i}")
                col = cols_sb[:, i:i + 1]
                nc.vector.tensor_scalar(a1[:], sqt[:], col, None, ALU.mult)
                nc.vector.tensor_scalar(a2[:], cqt[:], col, None, ALU.mult)
                a1s.append(a1)
                a2s.append(a2)

            kh = ladder(thk, FDK, "k")

            # ---------------- scores (interleaved batches) ----------------
            pscs = [ps_sc.tile([128, NKV], F32, tag=f"scores{b}", name=f"psc{b}")
                    for b in range(BPC)]
            for i in range(R):
                skt, ckt = kh[i]
                for b in range(BPC):
                    nc.tensor.matmul(
                        pscs[b][:],
                        a1s[i][:, b * NQ:(b + 1) * NQ],
                        ckt[:, b * NKV:(b + 1) * NKV],
                        start=(i == 0), stop=False, skip_group_check=True,
                    )
                    nc.tensor.matmul(
                        pscs[b][:],
                        a2s[i][:, b * NQ:(b + 1) * NQ],
                        skt[:, b * NKV:(b + 1) * NKV],
                        start=False, stop=False, skip_group_check=True,
                    )
            # mask addend: rank-1 K=1 term, MASK_NEG * (1 - valid)
            for b in range(BPC):
                nc.tensor.matmul(
                    pscs[b][:], mneg_sb[:, :], mrow_sb[:, b, :],
                    start=False, stop=True, skip_group_check=True,
                )

            # ---------------- softmax + out, per batch ----------------
            for b in range(BPC):
                psc = pscs[b]
                exp_sb = sp.tile([128, NKV], F32, tag="exp")
                den = sp.tile([128, 1], F32, tag="den")
                nc.scalar.activation(exp_sb[:], psc[:], AF.Exp, accum_out=den[:])
                rec = sp.tile([128, 1], F32, tag="rec")
                nc.vector.reciprocal(rec[:], den[:])
                attn_sb = sp.tile([128, NKV], F32, tag="attn")
                nc.vector.tensor_scalar(attn_sb[:], exp_sb[:], rec[:], None, ALU.mult)
                nc.sync.dma_start(attn[b, :, :], attn_sb[:])

                # attn^T via PE transposes, then out = attn^T.T @ values
                pst = ps_t.tile([128, NKV], F32, tag="attnT")
                for c in range(4):
                    nc.tensor.transpose(
                        pst[:, c * 128:(c + 1) * 128],
                        attn_sb[:, c * 128:(c + 1) * 128],
                        ident_sb[:],
                    )
                attnT = sp.tile([128, NKV], F32R, tag="attnT_sb")
                nc.vector.tensor_copy(attnT[:], pst[:])

                pso = ps_t.tile([128, VS], F32, tag="out")
                for c in range(4):
                    nc.tensor.matmul(
                        pso[:],
                        attnT[:, c * 128:(c + 1) * 128],
                        v_sb[:, b, c],
                        start=(c == 0), stop=(c == 3),
                    )
                out_sb = sp.tile([128, VS], F32, tag="out_sb")
                nc.vector.tensor_copy(out_sb[:], pso[:])
                nc.sync.dma_start(out[b, :, :], out_sb[:])

    nc.compile()
    return nc


_NC_CACHE = {}


def _get_nc():
    if "nc" not in _NC_CACHE:
        _NC_CACHE["nc"] = build_nc()
    return _NC_CACHE["nc"]


def _host_prep(queries, keys, values, valid_lens, Wq, Wk, wv):
    qT = np.ascontiguousarray(np.transpose(queries, (0, 2, 1)), dtype=np.float32)
    kT = np.ascontiguousarray(np.transpose(keys, (0, 2, 1)), dtype=np.float32)
    vals = np.ascontiguousarray(values, dtype=np.float32)
    vl = np.asarray(valid_lens)
    # (1 - mask) rows; paired with the constant MASK_NEG lhsT row (K=1 matmul)
    inv = (np.arange(NKV)[None, :] >= vl[:, None]).astype(np.float32)  # (B, NKV)
    mrow = inv.astype(ml_dtypes.bfloat16)
    mneg = np.full((1, NQ), MASK_NEG, dtype=ml_dtypes.bfloat16)
    sig = _sigma_list()
    cols = np.stack(
        [np.asarray(wv, np.float64) * BETA[i] * sig[i] for i in range(R)]
        + [np.full(H, HALF_PI)], axis=1
    ).astype(np.float32)  # (H, R+1); last col = pi/2 bias for the cos seeds
    ident = np.eye(128, dtype=np.float32)
    Wq = np.ascontiguousarray(Wq, dtype=np.float32)
    Wk = np.ascontiguousarray(Wk, dtype=np.float32)
    return qT, kT, vals, mrow, mneg, cols, ident, Wq, Wk


def run(inputs, trace=False):
    nc = _get_nc()
    qT, kT, vals, mrow, mneg, cols, ident, Wq_, Wk_ = _host_prep(**inputs)
    in_maps = []
    for core in range(NCORES):
        sl = slice(core * BPC, (core + 1) * BPC)
        in_maps.append({
            "qT": qT[sl], "kT": kT[sl], "vals": vals[sl],
            "mrow": np.ascontiguousarray(mrow[sl]).reshape(1, BPC, NKV),
            "mneg": mneg,
            "Wq": Wq_, "Wk": Wk_, "cols": cols, "ident": ident,
        })
    res = run_bass_kernel_spmd(nc, in_maps, list(range(NCORES)), trace=trace)
    out = np.concatenate([r["out"] for r in res.results], axis=0)
    attn = np.concatenate([r["attn"] for r in res.results], axis=0)
    return out.astype(np.float32), attn.astype(np.float32), res


def kernel(**inputs):
    out, attn, _ = run(inputs, trace=False)
    return out, attn


# revision 15
# speedup vs baseline: 1.0166x; 1.0166x over previous
"""Additive attention kernel for Trainium2 (8 NeuronCores, data-parallel over batch).

Math: out, attn = softmax_masked(einsum('bqkh,h', tanh(qWq[:,None]+kWk[None,:]), wv)) @ v

The O(B*NQ*NKV*H) tanh is evaluated via a separable sine expansion:
    tanh(x) ~= sum_m beta_m sin(w_m x),  w_m = base * 2^level  (octave ladder)
    sin(w(a+b)) = sin(wa)cos(wb) + cos(wa)sin(wb)
so scores become a sum of 2R rank-H matmuls on the tensor engine; the
per-element transcendentals run only on the small projected tensors.
Base sin/cos come from the ScalarE LUT (args stay within [-pi, pi]);
higher octaves use s2=s*c, c2=1-2s^2 (double angle) with power-of-two
scale factors folded into the per-term coefficient columns. Harmonics are
kept in bf16 (fast DVE modes + PE fast weight load); the -1e6 mask addend
rides in as one extra K=1 matmul term per batch.
"""

import sys

for _p in ("/opt/trn_rl_repo",):
    if _p not in sys.path:
        sys.path.insert(0, _p)

import numpy as np
import ml_dtypes

import concourse.bass as bass
import concourse.bacc as bacc
import concourse.tile as tile
from concourse import mybir
from concourse.bass_utils import run_bass_kernel_spmd

F32 = mybir.dt.float32
F32R = mybir.dt.float32r
BF16 = mybir.dt.bfloat16
AF = mybir.ActivationFunctionType
ALU = mybir.AluOpType

B, NQ, NKV = 16, 128, 512
QS, KS, VS, H = 256, 256, 256, 128
NCORES = 8
BPC = B // NCORES  # batches per core

# sine-ladder fit of tanh (Gaussian-weighted lstsq, see module docstring)
BASES = [0.2110214, 0.29999999]
LEVELS = [5, 5]
BETA = [
    0.89393810279, 0.21792006157, 0.20982351355, 0.08374484344, 0.011219404929,
    0.19989137427, 0.090749504641, 0.12138348077, 0.038702849552, 0.0018794616986,
]
R = sum(LEVELS)
HALF_PI = 1.5707963267948966
MASK_NEG = -60000.0  # large negative addend; exp(score + MASK_NEG) == 0 in fp32


def _sigma_list():
    """Per-frequency power-of-two factor: true s = sigma * stored s~."""
    sig = []
    for L in LEVELS:
        s = 1.0
        for _ in range(L):
            sig.append(s)
            s *= 2.0
    return sig


def build_nc():
    nc = bacc.Bacc(
        "TRN2", target_bir_lowering=False, debug=False, num_devices=NCORES
    )

    qT = nc.declare_dram_parameter("qT", [BPC, QS, NQ], BF16, False)
    kT = nc.declare_dram_parameter("kT", [BPC, KS, NKV], BF16, False)
    vals = nc.declare_dram_parameter("vals", [BPC, NKV, VS], BF16, False)
    Wq = nc.declare_dram_parameter("Wq", [QS, H], BF16, False)
    Wk = nc.declare_dram_parameter("Wk", [KS, H], BF16, False)
    cols = nc.declare_dram_parameter("cols", [H, R + 1], F32, False)
    mrow = nc.declare_dram_parameter("mrow", [1, BPC, NKV], BF16, False)
    mneg = nc.declare_dram_parameter("mneg", [1, NQ], BF16, False)
    ident = nc.declare_dram_parameter("ident", [128, 128], F32, False)
    out = nc.declare_dram_parameter("out", [BPC, NQ, VS], F32, isOutput=True)
    attn = nc.declare_dram_parameter("attn", [BPC, NQ, NKV], F32, isOutput=True)

    FDQ = BPC * NQ      # 256: fused q free dim [b, q]
    FDK = BPC * NKV     # 1024: fused k free dim [b, k]

    with tile.TileContext(nc) as tc:
        with (
            tc.tile_pool(name="const", bufs=1) as cp,
            tc.tile_pool(name="io", bufs=1) as iop,
            tc.tile_pool(name="harm", bufs=1) as hp,
            tc.tile_pool(name="tmp", bufs=3) as tp,
            tc.tile_pool(name="soft", bufs=2) as sp,
            tc.tile_pool(name="ps_proj", bufs=1, space="PSUM") as ps_proj,
            tc.tile_pool(name="ps_sc", bufs=1, space="PSUM") as ps_sc,
            tc.tile_pool(name="ps_t", bufs=1, space="PSUM") as ps_t,
        ):
            # ---------------- constants & inputs ----------------
            wq_sb = cp.tile([128, 2, H], BF16)
            wk_sb = cp.tile([128, 2, H], BF16)
            nc.sync.dma_start(wq_sb[:], Wq.rearrange("(c p) h -> p c h", p=128))
            nc.sync.dma_start(wk_sb[:], Wk.rearrange("(c p) h -> p c h", p=128))
            cols_sb = cp.tile([128, R + 1], F32)
            nc.scalar.dma_start(cols_sb[:], cols[:, :])
            ident_sb = cp.tile([128, 128], F32)
            nc.scalar.dma_start(ident_sb[:], ident[:, :])
            mrow_sb = cp.tile([1, BPC, NKV], BF16)
            nc.scalar.dma_start(mrow_sb[:], mrow[:, :, :])
            mneg_sb = cp.tile([1, NQ], BF16)
            nc.scalar.dma_start(mneg_sb[:], mneg[:, :])

            q_in = iop.tile([128, 2, BPC, NQ], BF16)   # [d-chunk][batch][q]
            k_in = iop.tile([128, 2, BPC, NKV], BF16)  # [d-chunk][batch][k]
            for b in range(BPC):
                nc.sync.dma_start(
                    q_in[:, :, b, :], qT[b].rearrange("(c p) q -> p c q", p=128))
                nc.sync.dma_start(
                    k_in[:, :, b, :], kT[b].rearrange("(c p) k -> p c k", p=128))
            v_sb = iop.tile([128, BPC, 4, VS], BF16)   # [batch][k-chunk][v]
            for b in range(BPC):
                nc.sync.dma_start(
                    v_sb[:, b, :, :], vals[b].rearrange("(c p) v -> p c v", p=128))

            # ---------------- projections: theta = x @ W ----------------
            thq_ps = ps_proj.tile([128, FDQ], F32)
            for c in range(2):
                nc.tensor.matmul(
                    thq_ps[:], wq_sb[:, c, :], q_in[:, c],
                    start=(c == 0), stop=(c == 1),
                )
            thq = iop.tile([128, FDQ], F32)
            nc.vector.tensor_copy(thq[:], thq_ps[:])

            thk_ps = ps_proj.tile([128, FDK], F32)
            for b in range(BPC):
                for c in range(2):
                    nc.tensor.matmul(
                        thk_ps[:, b * NKV:(b + 1) * NKV],
                        wk_sb[:, c, :], k_in[:, c, b],
                        start=(c == 0), stop=(c == 1),
                    )
            thk = iop.tile([128, FDK], F32)
            nc.vector.tensor_copy(thk[:], thk_ps[:])

            # ---------------- sine octave ladders (bf16 harmonics) --------
            def ladder(theta, FD, tag):
                tiles = []
                for bi, (nu, L) in enumerate(zip(BASES, LEVELS)):
                    s = hp.tile([128, FD], BF16, tag=f"{tag}s{bi}_0")
                    c = hp.tile([128, FD], BF16, tag=f"{tag}c{bi}_0")
                    nc.scalar.activation(s[:], theta[:], AF.Sin, scale=float(nu))
                    nc.scalar.activation(c[:], theta[:], AF.Sin,
                                         bias=cols_sb[:, R:R + 1],
                                         scale=float(nu))
                    tiles.append((s, c))
                    sigma = 1.0
                    for j in range(1, L):
                        spv, cpv = tiles[-1]
                        sq = tp.tile([128, FD], BF16, tag=f"{tag}sq")
                        nc.scalar.activation(sq[:], spv[:], AF.Square)
                        s2 = hp.tile([128, FD], BF16, tag=f"{tag}s{bi}_{j}")
                        nc.vector.tensor_mul(s2[:], spv[:], cpv[:])
                        c2 = hp.tile([128, FD], BF16, tag=f"{tag}c{bi}_{j}")
                        nc.vector.tensor_scalar(
                            c2[:], sq[:], -2.0 * sigma * sigma, 1.0,
                            ALU.mult, ALU.add,
                        )
                        sigma *= 2.0
                        tiles.append((s2, c2))
                return tiles

            # q-side first so the A-scalings (lhsT producers) run early on DVE
            qh = ladder(thq, FDQ, "q")

            # HAM warmup: junk matmuls on the first q-harmonic keep PE busy
            # from the ladder phase into the scores phase, so the scores
            # matmuls run at 2.4 GHz instead of the cold 1.2 GHz.
            warm_ps = ps_t.tile([128, NKV], F32, tag="warm")
            for w in range(20):
                nc.tensor.matmul(
                    warm_ps[:, 0:FDQ], qh[0][0][:, 0:128], qh[0][0][:],
                    start=True, stop=True, skip_group_check=True,
                )

            a1s, a2s = [], []
            for i in range(R):
                sqt, cqt = qh[i]
                a1 = hp.tile([128, FDQ], BF16, tag=f"a1_{i}")
                a2 = hp.tile([128, FDQ], BF16, tag=f"a2_{i}")
                col = cols_sb[:, i:i + 1]
                nc.vector.tensor_scalar(a1[:], sqt[:], col, None, ALU.mult)
                nc.vector.tensor_scalar(a2[:], cqt[:], col, None, ALU.mult)
                a1s.append(a1)
                a2s.append(a2)

            kh = ladder(thk, FDK, "k")

            # all Sin ops are issued; switch the ACT table to the exp set now
            # (Square/Copy are fillers present in every set) so the switch
            # overlaps the ladder instead of stalling the softmax.
            dummy = sp.tile([128, 1], F32, tag="dummy")
            nc.scalar.activation(dummy[:], cols_sb[:, 0:1], AF.Exp)

            # ---------------- scores (interleaved batches) ----------------
            pscs = [ps_sc.tile([128, NKV], F32, tag=f"scores{b}", name=f"psc{b}")
                    for b in range(BPC)]
            for i in range(R):
                skt, ckt = kh[i]
                for b in range(BPC):
                    nc.tensor.matmul(
                        pscs[b][:],
                        a1s[i][:, b * NQ:(b + 1) * NQ],
                        ckt[:, b * NKV:(b + 1) * NKV],
                        start=(i == 0), stop=False, skip_group_check=True,
                    )
                    nc.tensor.matmul(
                        pscs[b][:],
                        a2s[i][:, b * NQ:(b + 1) * NQ],
                        skt[:, b * NKV:(b + 1) * NKV],
                        start=False, stop=False, skip_group_check=True,
                    )
            # mask addend: rank-1 K=1 term, MASK_NEG * (1 - valid)
            for b in range(BPC):
                nc.tensor.matmul(
                    pscs[b][:], mneg_sb[:, :], mrow_sb[:, b, :],
                    start=False, stop=True, skip_group_check=True,
                )

            # ---------------- softmax + out, per batch ----------------
            for b in range(BPC):
                psc = pscs[b]
                exp_sb = sp.tile([128, NKV], F32, tag="exp")
                den = sp.tile([128, 1], F32, tag="den")
                nc.scalar.activation(exp_sb[:], psc[:], AF.Exp, accum_out=den[:])
                rec = sp.tile([128, 1], F32, tag="rec")
                nc.vector.reciprocal(rec[:], den[:])

                # exp^T via PE transposes; normalization is applied afterwards
                # (per-partition scale), keeping recip off the transpose path
                pst = ps_t.tile([128, NKV], F32, tag="expT")
                for c in range(4):
                    nc.tensor.transpose(
                        pst[:, c * 128:(c + 1) * 128],
                        exp_sb[:, c * 128:(c + 1) * 128],
                        ident_sb[:],
                    )
                expT = sp.tile([128, NKV], BF16, tag="expT_sb")
                nc.vector.tensor_copy(expT[:], pst[:])

                attn_sb = sp.tile([128, NKV], F32, tag="attn")
                nc.vector.tensor_scalar(attn_sb[:], exp_sb[:], rec[:], None, ALU.mult)
                nc.sync.dma_start(attn[b, :, :], attn_sb[:])

                pso = ps_t.tile([128, VS], F32, tag="out")
                for c in range(4):
                    nc.tensor.matmul(
                        pso[:],
                        expT[:, c * 128:(c + 1) * 128],
                        v_sb[:, b, c],
                        start=(c == 0), stop=(c == 3),
                    )
                out_sb = sp.tile([128, VS], F32, tag="out_sb")
                nc.vector.tensor_scalar(out_sb[:], pso[:], rec[:], None, ALU.mult)
                nc.sync.dma_start(out[b, :, :], out_sb[:])

    nc.compile()
    return nc


_NC_CACHE = {}


def _get_nc():
    if "nc" not in _NC_CACHE:
        _NC_CACHE["nc"] = build_nc()
    return _NC_CACHE["nc"]


def _host_prep(queries, keys, values, valid_lens, Wq, Wk, wv):
    qT = np.ascontiguousarray(
        np.transpose(np.asarray(queries, np.float32), (0, 2, 1))).astype(ml_dtypes.bfloat16)
    kT = np.ascontiguousarray(
        np.transpose(np.asarray(keys, np.float32), (0, 2, 1))).astype(ml_dtypes.bfloat16)
    vals = np.asarray(values, np.float32).astype(ml_dtypes.bfloat16)
    vl = np.asarray(valid_lens)
    # (1 - mask) rows; paired with the constant MASK_NEG lhsT row (K=1 matmul)
    inv = (np.arange(NKV)[None, :] >= vl[:, None]).astype(np.float32)  # (B, NKV)
    mrow = inv.astype(ml_dtypes.bfloat16)
    mneg = np.full((1, NQ), MASK_NEG, dtype=ml_dtypes.bfloat16)
    sig = _sigma_list()
    cols = np.stack(
        [np.asarray(wv, np.float64) * BETA[i] * sig[i] for i in range(R)]
        + [np.full(H, HALF_PI)], axis=1
    ).astype(np.float32)  # (H, R+1); last col = pi/2 bias for the cos seeds
    ident = np.eye(128, dtype=np.float32)
    Wq = np.asarray(Wq, np.float32).astype(ml_dtypes.bfloat16)
    Wk = np.asarray(Wk, np.float32).astype(ml_dtypes.bfloat16)
    return qT, kT, vals, mrow, mneg, cols, ident, Wq, Wk


def run(inputs, trace=False):
    nc = _get_nc()
    qT, kT, vals, mrow, mneg, cols, ident, Wq_, Wk_ = _host_prep(**inputs)
    in_maps = []
    for core in range(NCORES):
        sl = slice(core * BPC, (core + 1) * BPC)
        in_maps.append({
            "qT": qT[sl], "kT": kT[sl], "vals": vals[sl],
            "mrow": np.ascontiguousarray(mrow[sl]).reshape(1, BPC, NKV),
            "mneg": mneg,
            "Wq": Wq_, "Wk": Wk_, "cols": cols, "ident": ident,
        })
    res = run_bass_kernel_spmd(nc, in_maps, list(range(NCORES)), trace=trace)
    out = np.concatenate([r["out"] for r in res.results], axis=0)
    attn = np.concatenate([r["attn"] for r in res.results], axis=0)
    return out.astype(np.float32), attn.astype(np.float32), res


def kernel(**inputs):
    out, attn, _ = run(inputs, trace=False)
    return out, attn


# revision 17
# speedup vs baseline: 1.2037x; 1.1841x over previous
"""Additive attention kernel for Trainium2 (8 NeuronCores, data-parallel over batch).

Math: out, attn = softmax_masked(einsum('bqkh,h', tanh(qWq[:,None]+kWk[None,:]), wv)) @ v

The O(B*NQ*NKV*H) tanh is evaluated via a separable sine expansion:
    tanh(x) ~= sum_m beta_m sin(w_m x),  w_m = base * 2^level  (octave ladder)
    sin(w(a+b)) = sin(wa)cos(wb) + cos(wa)sin(wb)
so scores become a sum of 2R rank-H matmuls on the tensor engine; the
per-element transcendentals run only on the small projected tensors.
Base sin/cos come from the ScalarE LUT (args stay within [-pi, pi]);
higher octaves use s2=s*c, c2=1-2s^2 (double angle) with power-of-two
scale factors folded into the per-term coefficient columns. Harmonics are
kept in bf16 (fast DVE modes + PE fast weight load); the -1e6 mask addend
rides in as one extra K=1 matmul term per batch.
"""

import sys

for _p in ("/opt/trn_rl_repo",):
    if _p not in sys.path:
        sys.path.insert(0, _p)

import numpy as np
import ml_dtypes

import concourse.bass as bass
import concourse.bacc as bacc
import concourse.tile as tile
from concourse import mybir
from concourse.bass_utils import run_bass_kernel_spmd

F32 = mybir.dt.float32
F32R = mybir.dt.float32r
BF16 = mybir.dt.bfloat16
AF = mybir.ActivationFunctionType
ALU = mybir.AluOpType

B, NQ, NKV = 16, 128, 512
QS, KS, VS, H = 256, 256, 256, 128
NCORES = 8
BPC = B // NCORES  # batches per core

# sine-ladder fit of tanh (Gaussian-weighted lstsq, see module docstring)
BASES = [0.2110214, 0.29999999]
LEVELS = [5, 5]
BETA = [
    0.89393810279, 0.21792006157, 0.20982351355, 0.08374484344, 0.011219404929,
    0.19989137427, 0.090749504641, 0.12138348077, 0.038702849552, 0.0018794616986,
]
R = sum(LEVELS)
HALF_PI = 1.5707963267948966
MASK_NEG = -60000.0  # large negative addend; exp(score + MASK_NEG) == 0 in fp32


def _sigma_list():
    """Per-frequency power-of-two factor: true s = sigma * stored s~."""
    sig = []
    for L in LEVELS:
        s = 1.0
        for _ in range(L):
            sig.append(s)
            s *= 2.0
    return sig


def build_nc():
    nc = bacc.Bacc(
        "TRN2", target_bir_lowering=False, debug=False, num_devices=NCORES
    )

    qT = nc.declare_dram_parameter("qT", [BPC, QS, NQ], BF16, False)
    kT = nc.declare_dram_parameter("kT", [BPC, KS, NKV], BF16, False)
    vals = nc.declare_dram_parameter("vals", [BPC, NKV, VS], BF16, False)
    Wq = nc.declare_dram_parameter("Wq", [QS, H], BF16, False)
    Wk = nc.declare_dram_parameter("Wk", [KS, H], BF16, False)
    cols = nc.declare_dram_parameter("cols", [H, R + 1], F32, False)
    mrow = nc.declare_dram_parameter("mrow", [1, BPC, NKV], BF16, False)
    mneg = nc.declare_dram_parameter("mneg", [1, NQ], BF16, False)
    ident = nc.declare_dram_parameter("ident", [128, 128], F32, False)
    out = nc.declare_dram_parameter("out", [BPC, NQ, VS], F32, isOutput=True)
    attn = nc.declare_dram_parameter("attn", [BPC, NQ, NKV], F32, isOutput=True)

    FDQ = BPC * NQ      # 256: fused q free dim [b, q]
    FDK = BPC * NKV     # 1024: fused k free dim [b, k]

    with tile.TileContext(nc) as tc:
        with (
            tc.tile_pool(name="const", bufs=1) as cp,
            tc.tile_pool(name="io", bufs=1) as iop,
            tc.tile_pool(name="harm", bufs=1) as hp,
            tc.tile_pool(name="tmp", bufs=3) as tp,
            tc.tile_pool(name="soft", bufs=2) as sp,
            tc.tile_pool(name="ps_proj", bufs=1, space="PSUM") as ps_proj,
            tc.tile_pool(name="ps_sc", bufs=1, space="PSUM") as ps_sc,
            tc.tile_pool(name="ps_t", bufs=1, space="PSUM") as ps_t,
        ):
            # ---------------- constants & inputs ----------------
            wq_sb = cp.tile([128, 2, H], BF16)
            wk_sb = cp.tile([128, 2, H], BF16)
            nc.sync.dma_start(wq_sb[:], Wq.rearrange("(c p) h -> p c h", p=128))
            nc.sync.dma_start(wk_sb[:], Wk.rearrange("(c p) h -> p c h", p=128))
            cols_sb = cp.tile([128, R + 1], F32)
            nc.scalar.dma_start(cols_sb[:], cols[:, :])
            ident_sb = cp.tile([128, 128], F32)
            nc.scalar.dma_start(ident_sb[:], ident[:, :])
            mrow_sb = cp.tile([1, BPC, NKV], BF16)
            nc.scalar.dma_start(mrow_sb[:], mrow[:, :, :])
            mneg_sb = cp.tile([1, NQ], BF16)
            nc.scalar.dma_start(mneg_sb[:], mneg[:, :])

            q_in = iop.tile([128, 2, BPC, NQ], BF16)   # [d-chunk][batch][q]
            k_in = iop.tile([128, 2, BPC, NKV], BF16)  # [d-chunk][batch][k]
            for b in range(BPC):
                nc.sync.dma_start(
                    q_in[:, :, b, :], qT[b].rearrange("(c p) q -> p c q", p=128))
                nc.sync.dma_start(
                    k_in[:, :, b, :], kT[b].rearrange("(c p) k -> p c k", p=128))
            v_sb = iop.tile([128, BPC, 4, VS], BF16)   # [batch][k-chunk][v]
            for b in range(BPC):
                nc.sync.dma_start(
                    v_sb[:, b, :, :], vals[b].rearrange("(c p) v -> p c v", p=128))

            # ---------------- projections: theta = x @ W ----------------
            thq_ps = ps_proj.tile([128, FDQ], F32)
            for c in range(2):
                nc.tensor.matmul(
                    thq_ps[:], wq_sb[:, c, :], q_in[:, c],
                    start=(c == 0), stop=(c == 1),
                )

            thk_ps = ps_proj.tile([128, FDK], F32)
            for b in range(BPC):
                for c in range(2):
                    nc.tensor.matmul(
                        thk_ps[:, b * NKV:(b + 1) * NKV],
                        wk_sb[:, c, :], k_in[:, c, b],
                        start=(c == 0), stop=(c == 1),
                    )

            # ---------------- sine octave ladders (bf16 harmonics) --------
            def ladder(theta, FD, tag, sq_on_act):
                tiles = []
                for bi, (nu, L) in enumerate(zip(BASES, LEVELS)):
                    s = hp.tile([128, FD], BF16, tag=f"{tag}s{bi}_0")
                    c = hp.tile([128, FD], BF16, tag=f"{tag}c{bi}_0")
                    nc.scalar.activation(s[:], theta[:], AF.Sin, scale=float(nu))
                    nc.scalar.activation(c[:], theta[:], AF.Sin,
                                         bias=cols_sb[:, R:R + 1],
                                         scale=float(nu))
                    tiles.append((s, c))
                    sigma = 1.0
                    for j in range(1, L):
                        spv, cpv = tiles[-1]
                        sq = tp.tile([128, FD], BF16, tag=f"{tag}sq")
                        if sq_on_act:
                            nc.scalar.activation(sq[:], spv[:], AF.Square)
                        else:
                            nc.vector.tensor_mul(sq[:], spv[:], spv[:])
                        s2 = hp.tile([128, FD], BF16, tag=f"{tag}s{bi}_{j}")
                        nc.vector.tensor_mul(s2[:], spv[:], cpv[:])
                        c2 = hp.tile([128, FD], BF16, tag=f"{tag}c{bi}_{j}")
                        nc.vector.tensor_scalar(
                            c2[:], sq[:], -2.0 * sigma * sigma, 1.0,
                            ALU.mult, ALU.add,
                        )
                        sigma *= 2.0
                        tiles.append((s2, c2))
                return tiles

            # q-side first so the A-scalings (lhsT producers) run early on DVE
            qh = ladder(thq_ps, FDQ, "q", sq_on_act=False)

            # HAM warmup: junk matmuls on the first q-harmonic keep PE busy
            # from the ladder phase into the scores phase, so the scores
            # matmuls run at 2.4 GHz instead of the cold 1.2 GHz.
            warm_ps = ps_t.tile([128, NKV], F32, tag="warm")
            for w in range(20):
                nc.tensor.matmul(
                    warm_ps[:, 0:FDQ], qh[0][0][:, 0:128], qh[0][0][:],
                    start=True, stop=True, skip_group_check=True,
                )

            a1s, a2s = [], []
            for i in range(R):
                sqt, cqt = qh[i]
                a1 = hp.tile([128, FDQ], BF16, tag=f"a1_{i}")
                a2 = hp.tile([128, FDQ], BF16, tag=f"a2_{i}")
                col = cols_sb[:, i:i + 1]
                nc.vector.tensor_scalar(a1[:], sqt[:], col, None, ALU.mult)
                nc.vector.tensor_scalar(a2[:], cqt[:], col, None, ALU.mult)
                a1s.append(a1)
                a2s.append(a2)

            kh = ladder(thk_ps, FDK, "k", sq_on_act=True)

            # all Sin ops are issued; switch the ACT table to the exp set now
            # (Square/Copy are fillers present in every set) so the switch
            # overlaps the ladder instead of stalling the softmax.
            dummy = sp.tile([128, 1], F32, tag="dummy")
            nc.scalar.activation(dummy[:], kh[LEVELS[0]][1][:, 0:1], AF.Exp)

            # ---------------- scores (interleaved batches) ----------------
            pscs = [ps_sc.tile([128, NKV], F32, tag=f"scores{b}", name=f"psc{b}")
                    for b in range(BPC)]
            for i in range(R):
                skt, ckt = kh[i]
                for b in range(BPC):
                    nc.tensor.matmul(
                        pscs[b][:],
                        a1s[i][:, b * NQ:(b + 1) * NQ],
                        ckt[:, b * NKV:(b + 1) * NKV],
                        start=(i == 0), stop=False, skip_group_check=True,
                    )
                    nc.tensor.matmul(
                        pscs[b][:],
                        a2s[i][:, b * NQ:(b + 1) * NQ],
                        skt[:, b * NKV:(b + 1) * NKV],
                        start=False, stop=False, skip_group_check=True,
                    )
            # mask addend: rank-1 K=1 term, MASK_NEG * (1 - valid)
            for b in range(BPC):
                nc.tensor.matmul(
                    pscs[b][:], mneg_sb[:, :], mrow_sb[:, b, :],
                    start=False, stop=True, skip_group_check=True,
                )

            # ---------------- softmax + out, per batch ----------------
            for b in range(BPC):
                psc = pscs[b]
                exp_sb = sp.tile([128, NKV], F32, tag="exp")
                den = sp.tile([128, 1], F32, tag="den")
                nc.scalar.activation(exp_sb[:], psc[:], AF.Exp, accum_out=den[:])
                rec = sp.tile([128, 1], F32, tag="rec")
                nc.vector.reciprocal(rec[:], den[:])

                # exp^T via PE transposes; normalization is applied afterwards
                # (per-partition scale), keeping recip off the transpose path
                pst = ps_t.tile([128, NKV], F32, tag="expT")
                for c in range(4):
                    nc.tensor.transpose(
                        pst[:, c * 128:(c + 1) * 128],
                        exp_sb[:, c * 128:(c + 1) * 128],
                        ident_sb[:],
                    )
                expT = sp.tile([128, NKV], BF16, tag="expT_sb")
                nc.vector.tensor_copy(expT[:], pst[:])

                attn_sb = sp.tile([128, NKV], F32, tag="attn")
                nc.vector.tensor_scalar(attn_sb[:], exp_sb[:], rec[:], None, ALU.mult)
                nc.sync.dma_start(attn[b, :, :], attn_sb[:])

                pso = ps_t.tile([128, VS], F32, tag="out")
                for c in range(4):
                    nc.tensor.matmul(
                        pso[:],
                        expT[:, c * 128:(c + 1) * 128],
                        v_sb[:, b, c],
                        start=(c == 0), stop=(c == 3),
                    )
                out_sb = sp.tile([128, VS], F32, tag="out_sb")
                nc.vector.tensor_scalar(out_sb[:], pso[:], rec[:], None, ALU.mult)
                nc.sync.dma_start(out[b, :, :], out_sb[:])

    nc.compile()
    return nc


_NC_CACHE = {}


def _get_nc():
    if "nc" not in _NC_CACHE:
        _NC_CACHE["nc"] = build_nc()
    return _NC_CACHE["nc"]


def _host_prep(queries, keys, values, valid_lens, Wq, Wk, wv):
    qT = np.ascontiguousarray(
        np.transpose(np.asarray(queries, np.float32), (0, 2, 1))).astype(ml_dtypes.bfloat16)
    kT = np.ascontiguousarray(
        np.transpose(np.asarray(keys, np.float32), (0, 2, 1))).astype(ml_dtypes.bfloat16)
    vals = np.asarray(values, np.float32).astype(ml_dtypes.bfloat16)
    vl = np.asarray(valid_lens)
    # (1 - mask) rows; paired with the constant MASK_NEG lhsT row (K=1 matmul)
    inv = (np.arange(NKV)[None, :] >= vl[:, None]).astype(np.float32)  # (B, NKV)
    mrow = inv.astype(ml_dtypes.bfloat16)
    mneg = np.full((1, NQ), MASK_NEG, dtype=ml_dtypes.bfloat16)
    sig = _sigma_list()
    cols = np.stack(
        [np.asarray(wv, np.float64) * BETA[i] * sig[i] for i in range(R)]
        + [np.full(H, HALF_PI)], axis=1
    ).astype(np.float32)  # (H, R+1); last col = pi/2 bias for the cos seeds
    ident = np.eye(128, dtype=np.float32)
    Wq = np.asarray(Wq, np.float32).astype(ml_dtypes.bfloat16)
    Wk = np.asarray(Wk, np.float32).astype(ml_dtypes.bfloat16)
    return qT, kT, vals, mrow, mneg, cols, ident, Wq, Wk


def run(inputs, trace=False):
    nc = _get_nc()
    qT, kT, vals, mrow, mneg, cols, ident, Wq_, Wk_ = _host_prep(**inputs)
    in_maps = []
    for core in range(NCORES):
        sl = slice(core * BPC, (core + 1) * BPC)
        in_maps.append({
            "qT": qT[sl], "kT": kT[sl], "vals": vals[sl],
            "mrow": np.ascontiguousarray(mrow[sl]).reshape(1, BPC, NKV),
            "mneg": mneg,
            "Wq": Wq_, "Wk": Wk_, "cols": cols, "ident": ident,
        })
    res = run_bass_kernel_spmd(nc, in_maps, list(range(NCORES)), trace=trace)
    out = np.concatenate([r["out"] for r in res.results], axis=0)
    attn = np.concatenate([r["attn"] for r in res.results], axis=0)
    return out.astype(np.float32), attn.astype(np.float32), res


def kernel(**inputs):
    out, attn, _ = run(inputs, trace=False)
    return out, attn
